# revision 3
# baseline (speedup 1.0000x reference)
"""Causal attention (B=8, N=4096 flattened 64x64, d=128) on 8 trn2 cores.

Sharding: data-parallel over batch -- core b gets batch element b.

Per-core algorithm (flash-style, transposed orientation):
  inputs per core (host pre-transposed):
    qT [128, 4096] bf16  (c on partitions, query pos on free)
    kT [128, 4096] bf16
    vT [128, 4096] bf16  (k-within-tile on partitions: vT[p, 128j+c] = v[128j+p, c])
  loop q-chunks of 512, k-tiles of 128 (j = 0..4t+3):
    S^T[k, q] = kT_j.T @ qT_chunk          (PE, PSUM, N=512, bf16 moving)
    E = exp(S^T / sqrt(128))  -> bf16      (ScalarE, PSUM->SBUF, groups of 3 j)
    causal mask on diagonal tiles          (GpSimd affine_select, fill 0)
    O^T += v_j.T @ E_j                     (PE, accumulate in PSUM over j)
    denom[q] += sum_k E_j[k, q]            (split: PE all-ones matmul / DVE adds)
  Diagonal k-tiles narrow their S/PV matmuls to the non-masked column range;
  the skipped PSUM prefix holds garbage, exp of it is zero-filled by the
  affine_select (select semantics, so inf/NaN get dropped, not multiplied).
  outputs per core: outT [128, 4096] (unnormalized O^T), den [1, 4096]
  host: out = (outT / den).T

Chunks are processed in order 1..7,0 so the tail (last exp -> PV -> copy ->
DMA) is the smallest chunk. Inputs arrive as 4 independent pieces per tensor
(separate SBUF tiles, so the piece DMAs don't serialize on WAW tracking),
spread across the scalar/vector/sync/gpsimd queues, widest pieces last.

No max-subtraction in softmax: scores are ~N(0,1) (max |s| < ~7), exp is safe
in fp32 and softmax is shift-invariant. Masked probabilities are exactly zero
(select with fill=0), matching the reference's `softmax(.)*allowed`.
"""

import math

import ml_dtypes
import numpy as np

import concourse.bacc as bacc
import concourse.mybir as mybir
import concourse.tile as tile
from concourse.bass import ts, ds
from concourse.bass_utils import run_bass_kernel_spmd

P = 128
NSEQ = 4096
QCH = 512              # query positions per chunk
NCH = NSEQ // QCH      # 8 chunks
GROUP = 3              # k-tiles per exp group (3 PSUM banks; x2 buffered)
SCALE = 1.0 / math.sqrt(128.0)
F32 = mybir.dt.float32
BF16 = mybir.dt.bfloat16
N_CORES = 8
PE_DEN_MOD = 2         # every PE_DEN_MOD groups -> denominator matmul on PE

CHUNK_ORDER = [1, 2, 3, 4, 5, 6, 7, 0]   # end on the smallest chunk
# input pieces (column ranges) per tensor; piece 0 small for fast start
PIECES = [(0, 512), (512, 1536), (1536, 2560), (2560, 4096)]

_nc_cache = []


def _build():
    nc = bacc.Bacc("TRN2", target_bir_lowering=False, debug=False,
                   num_devices=N_CORES)
    qT = nc.dram_tensor("qT", [P, NSEQ], BF16, kind="ExternalInput").ap()
    kT = nc.dram_tensor("kT", [P, NSEQ], BF16, kind="ExternalInput").ap()
    vT = nc.dram_tensor("vT", [P, NSEQ], BF16, kind="ExternalInput").ap()
    outT = nc.dram_tensor("outT", [P, NSEQ], F32, kind="ExternalOutput").ap()
    den = nc.dram_tensor("den", [1, NSEQ], F32, kind="ExternalOutput").ap()

    exp_fn = mybir.ActivationFunctionType.Exp
    is_ge = mybir.AluOpType.is_ge

    with tile.TileContext(nc) as tc:
        with (
            tc.tile_pool(name="const", bufs=1) as cpool,
            tc.tile_pool(name="epool", bufs=13) as epool,
            tc.tile_pool(name="qpool", bufs=12) as qpool,
            tc.tile_pool(name="spool", bufs=2) as spool,
            tc.tile_pool(name="ps_s", bufs=2, space="PSUM") as ps_pool,
            tc.tile_pool(name="ps_o", bufs=1, space="PSUM") as po_pool,
            tc.tile_pool(name="ps_d", bufs=1, space="PSUM") as pd_pool,
        ):
            ones_sq = cpool.tile([P, P], BF16)
            nc.gpsimd.memset(ones_sq, 1.0)
            # pre-warm the PE during the input-DMA wait so the HAM clock
            # gate is at 2.4 GHz when real work starts; chunk order [0]'s
            # first denominator matmul clears the db bank anyway
            warm_db = pd_pool.tile([P, QCH], F32, tag="db", name="warm")
            for wi in range(40):
                nc.tensor.matmul(warm_db[:, ds(0, 64)], ones_sq,
                                 ones_sq[:, :64], start=True, stop=True)

            # input pieces: separate tiles so their DMAs are independent
            # (a single destination tile serializes the piece DMAs WAW).
            # Queue split keeps piece-0 triggers first on each queue.
            kp, qp, vp = [], [], []
            for pi, (c0, c1) in enumerate(PIECES):
                w = c1 - c0
                kp.append(cpool.tile([P, w], BF16, name=f"kp{pi}"))
                qp.append(cpool.tile([P, w], BF16, name=f"qp{pi}"))
                vp.append(cpool.tile([P, w], BF16, name=f"vp{pi}"))
            for pi in (0, 1):           # scalar queue is free pre-exp
                c0, c1 = PIECES[pi]
                nc.scalar.dma_start(kp[pi], kT[:, ds(c0, c1 - c0)])
            for pi in (0, 1, 2, 3):
                c0, c1 = PIECES[pi]
                nc.gpsimd.dma_start(vp[pi], vT[:, ds(c0, c1 - c0)])
            # sync queue: chunk 1's q piece first, chunk 0's (processed
            # last) at the end; k tails interleaved by first use
            for tname, pi in (("q", 1), ("k", 2), ("q", 2), ("k", 3),
                              ("q", 3), ("q", 0)):
                c0, c1 = PIECES[pi]
                dst, src = (qp[pi], qT) if tname == "q" else (kp[pi], kT)
                nc.sync.dma_start(dst, src[:, ds(c0, c1 - c0)])

            def piece_of(col):
                for pi, (c0, c1) in enumerate(PIECES):
                    if c0 <= col < c1:
                        return pi, c0
                raise AssertionError(col)

            def k_tile(j):
                pi, c0 = piece_of(j * P)
                return kp[pi][:, ds(j * P - c0, P)]

            def v_tile(j):
                pi, c0 = piece_of(j * P)
                return vp[pi][:, ds(j * P - c0, P)]

            def q_chunk(t):
                pi, c0 = piece_of(t * QCH)
                return qp[pi][:, ds(t * QCH - c0, QCH)]

            def emit_pv(job):
                # deferred PV + denominator matmuls for one group
                # (software pipelining: keeps the in-order PE queue from
                # head-of-line blocking on the exp/select chain of the group)
                (t, j0, gn, nj, e_sb, o_ps, db_ps, den_blk,
                 den_first, den_last) = job
                for d in range(gn):
                    j = j0 + d
                    dd = j - 4 * t
                    off = max(dd, 0) * P
                    nc.tensor.matmul(
                        o_ps[:, ds(off, QCH - off)],
                        v_tile(j),
                        e_sb[:, ds(d * QCH + off, QCH - off)],
                        start=(j == 0), stop=(j == nj - 1))
                if den_blk is not None:
                    nc.tensor.matmul(db_ps, ones_sq, den_blk,
                                     start=den_first, stop=den_last)
                if j0 + gn == nj:      # last group: flush chunk outputs
                    out_sb = spool.tile([P, QCH], F32, tag="osb",
                                        name=f"osb{t}")
                    den_sb = spool.tile([1, QCH], F32, tag="den",
                                        name=f"den{t}")
                    if t == CHUNK_ORDER[-1]:   # tail: split across engines
                        nc.scalar.copy(out_sb, o_ps)
                        nc.vector.tensor_copy(den_sb, db_ps[0:1, :])
                    else:
                        nc.vector.tensor_copy(out_sb, o_ps)
                        nc.vector.tensor_copy(den_sb, db_ps[0:1, :])
                    nc.sync.dma_start(outT[:, ts(t, QCH)], out_sb)
                    nc.sync.dma_start(den[:, ts(t, QCH)], den_sb)

            pv_pending = None
            for t in CHUNK_ORDER:
                nj = 4 * (t + 1)          # causal: k-tiles 0..4t+3
                q_sl = q_chunk(t)
                o_ps = po_pool.tile([P, QCH], F32, tag="o")
                db_ps = pd_pool.tile([P, QCH], F32, tag="db")
                den_carry = None
                den_count = 0

                groups = []
                j0 = 0
                while j0 < nj:
                    gn = min(GROUP, nj - j0)
                    groups.append((j0, gn))
                    j0 += gn

                for (j0, gn) in groups:
                    s_ps = ps_pool.tile([P, gn * QCH], F32, tag="s",
                                        padded_shape=[P, GROUP * QCH])
                    for d in range(gn):
                        j = j0 + d
                        dd = j - 4 * t
                        off = max(dd, 0) * P   # fully-masked column prefix
                        nc.tensor.matmul(
                            s_ps[:, ds(d * QCH + off, QCH - off)],
                            k_tile(j), q_sl[:, ds(off, QCH - off)],
                            start=True, stop=True)
                    e_sb = epool.tile([P, gn * QCH], BF16, tag="e",
                                      padded_shape=[P, GROUP * QCH])
                    nc.scalar.activation(e_sb, s_ps, exp_fn, scale=SCALE)

                    # causal mask on diagonal tiles (j in [4t, 4t+4)):
                    # keep where qcol - k - 128*dd >= 0, else fill 0.
                    # Covers the skipped S prefix too (exp of stale PSUM).
                    for d in range(gn):
                        j = j0 + d
                        dd = j - 4 * t
                        if dd >= 0:
                            w = (dd + 1) * P
                            reg = e_sb[:, ds(d * QCH, w)]
                            nc.gpsimd.affine_select(
                                out=reg, in_=reg, compare_op=is_ge,
                                fill=0.0, base=-dd * P, pattern=[[1, w]],
                                channel_multiplier=-1)

                    # denominator partials: sum blocks on DVE (bf16 2x
                    # adds), chaining across pairs of groups; one all-ones
                    # matmul per pair reduces over partitions into db
                    gidx = j0 // GROUP
                    chain = den_carry if gidx % 2 == 1 else None
                    if gn == 1 and chain is None:
                        den_blk = e_sb[:, :QCH]
                    else:
                        qacc = qpool.tile([P, QCH], BF16, tag="qacc")
                        first2 = (chain if chain is not None
                                  else e_sb[:, ts(1, QCH)])
                        nc.vector.tensor_add(qacc, e_sb[:, ts(0, QCH)],
                                             first2)
                        for d in range(1 if chain is not None else 2, gn):
                            nc.vector.tensor_add(qacc, qacc,
                                                 e_sb[:, ts(d, QCH)])
                        den_blk = qacc
                    if gidx % 2 == 0 and j0 + gn < nj:
                        den_carry = den_blk      # defer to next group
                        den_blk = None
                    else:
                        den_carry = None

                    if pv_pending is not None:
                        emit_pv(pv_pending)
                    den_first = den_blk is not None and den_count == 0
                    den_last = j0 + gn == nj
                    if den_blk is not None:
                        den_count += 1
                    pv_pending = (t, j0, gn, nj, e_sb, o_ps, db_ps, den_blk,
                                  den_first, den_last)

            emit_pv(pv_pending)

    nc.compile()
    return nc


def _get_nc():
    if not _nc_cache:
        _nc_cache.append(_build())
    return _nc_cache[0]


def _prep(query, key, value):
    B, H, W, C = query.shape
    CV = value.shape[-1]
    n = H * W
    q = (np.asarray(query, np.float32).reshape(B, n, C).transpose(0, 2, 1)
         .astype(ml_dtypes.bfloat16))
    q = np.ascontiguousarray(q)
    k = np.ascontiguousarray(
        np.asarray(key, np.float32).reshape(B, n, C).transpose(0, 2, 1)
        .astype(ml_dtypes.bfloat16))
    # vT[b, p, 128j+c] = v[b, 128j+p, c]: k-within-tile on partitions, so a
    # [128, 128] SBUF slice is directly the PV weight tile, and the HBM
    # lines are long and contiguous (8 KB per partition row)
    v = (np.asarray(value, np.float32).reshape(B, n // P, P, CV)
         .transpose(0, 2, 1, 3).reshape(B, P, n // P * CV)
         .astype(ml_dtypes.bfloat16))
    v = np.ascontiguousarray(v)
    return q, k, v


def kernel(query, key, value):
    B, H, W, C = query.shape
    CV = value.shape[-1]
    n = H * W
    q, k, v = _prep(query, key, value)

    nc = _get_nc()
    in_maps = [{"qT": q[b], "kT": k[b], "vT": v[b]} for b in range(B)]
    res = run_bass_kernel_spmd(nc, in_maps, core_ids=list(range(N_CORES)))

    out = np.empty((B, n, CV), np.float32)
    for b in range(B):
        oT = res.results[b]["outT"]          # [128, 4096] unnormalized O^T
        dn = res.results[b]["den"]           # [1, 4096]
        out[b] = (oT / dn).T
    return out.reshape(B, H, W, CV)


# revision 8
# speedup vs baseline: 1.0824x; 1.0824x over previous
"""Causal attention (B=8, N=4096 flattened 64x64, d=128) on 8 trn2 cores.

Sharding: data-parallel over batch -- core b gets batch element b.

Per-core algorithm (flash-style, transposed orientation):
  inputs per core (host pre-transposed):
    qT [128, 4096] bf16  (c on partitions, query pos on free)
    kT [128, 4096] bf16
    vT [128, 4096] bf16  (k-within-tile on partitions: vT[p, 128j+c] = v[128j+p, c])
  loop q-chunks of 512, k-tiles of 128 (j = 0..4t+3):
    S^T[k, q] = kT_j.T @ qT_chunk          (PE, PSUM, N=512, bf16 moving)
    E = exp(S^T / sqrt(128))  -> bf16      (ScalarE, PSUM->SBUF, groups of 3 j)
    causal mask on diagonal tiles          (GpSimd affine_select, fill 0)
    O^T += v_j.T @ E_j                     (PE, accumulate in PSUM over j)
    denom[q] += sum_k E_j[k, q]            (split: PE all-ones matmul / DVE adds)
  Diagonal k-tiles narrow their S/PV matmuls to the non-masked column range;
  the skipped PSUM prefix holds garbage, exp of it is zero-filled by the
  affine_select (select semantics, so inf/NaN get dropped, not multiplied).
  outputs per core: outT [128, 4096] (unnormalized O^T), den [1, 4096]
  host: out = (outT / den).T

Chunks are processed in order 1..7,0 so the tail (last exp -> PV -> copy ->
DMA) is the smallest chunk. Inputs arrive as 4 independent pieces per tensor
(separate SBUF tiles, so the piece DMAs don't serialize on WAW tracking),
spread across the scalar/vector/sync/gpsimd queues, widest pieces last.

No max-subtraction in softmax: scores are ~N(0,1) (max |s| < ~7), exp is safe
in fp32 and softmax is shift-invariant. Masked probabilities are exactly zero
(select with fill=0), matching the reference's `softmax(.)*allowed`.
"""

import math

import ml_dtypes
import numpy as np

import concourse.bacc as bacc
import concourse.mybir as mybir
import concourse.tile as tile
from concourse.bass import ts, ds
from concourse.bass_utils import run_bass_kernel_spmd

P = 128
NSEQ = 4096
QCH = 512              # query positions per chunk
NCH = NSEQ // QCH      # 8 chunks
GROUP = 3              # k-tiles per exp group (3 PSUM banks; x2 buffered)
SCALE = 1.0 / math.sqrt(128.0)
F32 = mybir.dt.float32
BF16 = mybir.dt.bfloat16
N_CORES = 8
PE_DEN_MOD = 2         # every PE_DEN_MOD groups -> denominator matmul on PE

CHUNK_ORDER = [0, 2, 3, 4, 5, 6, 7, 1]   # start AND end on small chunks
# input pieces (column ranges) per tensor; piece 0 small for fast start
PIECES = [(0, 512), (512, 1536), (1536, 2560), (2560, 4096)]

_nc_cache = []


def _build():
    nc = bacc.Bacc("TRN2", target_bir_lowering=False, debug=False,
                   num_devices=N_CORES)
    qT = nc.dram_tensor("qT", [P, NSEQ], BF16, kind="ExternalInput").ap()
    kT = nc.dram_tensor("kT", [P, NSEQ], BF16, kind="ExternalInput").ap()
    vT = nc.dram_tensor("vT", [P, NSEQ], BF16, kind="ExternalInput").ap()
    outT = nc.dram_tensor("outT", [P, NSEQ], F32, kind="ExternalOutput").ap()
    den = nc.dram_tensor("den", [1, NSEQ], F32, kind="ExternalOutput").ap()

    exp_fn = mybir.ActivationFunctionType.Exp
    is_ge = mybir.AluOpType.is_ge

    with tile.TileContext(nc) as tc:
        with (
            tc.tile_pool(name="const", bufs=1) as cpool,
            tc.tile_pool(name="epool", bufs=13) as epool,
            tc.tile_pool(name="qpool", bufs=12) as qpool,
            tc.tile_pool(name="spool", bufs=2) as spool,
            tc.tile_pool(name="ps_s", bufs=2, space="PSUM") as ps_pool,
            tc.tile_pool(name="ps_o", bufs=1, space="PSUM") as po_pool,
            tc.tile_pool(name="ps_d", bufs=1, space="PSUM") as pd_pool,
        ):
            ones_sq = cpool.tile([P, P], BF16)
            nc.gpsimd.memset(ones_sq, 1.0)
            # pre-warm the PE during the input-DMA wait so the HAM clock
            # gate is at 2.4 GHz when real work starts; chunk order [0]'s
            # first denominator matmul clears the db bank anyway
            warm_db = pd_pool.tile([P, QCH], F32, tag="db", name="warm")
            for wi in range(16):
                nc.tensor.matmul(warm_db[:, ds(0, 64)], ones_sq,
                                 ones_sq[:, :64], start=True, stop=True)

            # input pieces: separate tiles so their DMAs are independent
            # (a single destination tile serializes the piece DMAs WAW).
            # Queue split keeps piece-0 triggers first on each queue.
            kp, qp, vp = [], [], []
            for pi, (c0, c1) in enumerate(PIECES):
                w = c1 - c0
                kp.append(cpool.tile([P, w], BF16, name=f"kp{pi}"))
                qp.append(cpool.tile([P, w], BF16, name=f"qp{pi}"))
                vp.append(cpool.tile([P, w], BF16, name=f"vp{pi}"))
            # piece 0 of each tensor split into partition halves across
            # queues for an early-bandwidth boost; the remaining pieces
            # are ordered by first use (chunk order 0,2,3,...,1)
            half = ds(0, 512)
            nc.scalar.dma_start(kp[0][0:64, :], kT[0:64, half])
            nc.scalar.dma_start(kp[0][64:128, :], kT[64:128, half])
            nc.gpsimd.dma_start(vp[0][0:64, :], vT[0:64, half])
            nc.gpsimd.dma_start(vp[0][64:128, :], vT[64:128, half])
            nc.sync.dma_start(qp[0][0:64, :], qT[0:64, half])
            nc.sync.dma_start(qp[0][64:128, :], qT[64:128, half])
            c0, c1 = PIECES[1]
            nc.scalar.dma_start(kp[1], kT[:, ds(c0, c1 - c0)])
            for tname, pi in (("q", 1), ("v", 1), ("q", 2), ("k", 2),
                              ("v", 2), ("q", 3), ("k", 3), ("v", 3)):
                c0, c1 = PIECES[pi]
                dst, src = {"q": (qp[pi], qT), "k": (kp[pi], kT),
                            "v": (vp[pi], vT)}[tname]
                nc.sync.dma_start(dst, src[:, ds(c0, c1 - c0)])

            def piece_of(col):
                for pi, (c0, c1) in enumerate(PIECES):
                    if c0 <= col < c1:
                        return pi, c0
                raise AssertionError(col)

            def k_tile(j):
                pi, c0 = piece_of(j * P)
                return kp[pi][:, ds(j * P - c0, P)]

            def v_tile(j):
                pi, c0 = piece_of(j * P)
                return vp[pi][:, ds(j * P - c0, P)]

            def q_chunk(t):
                pi, c0 = piece_of(t * QCH)
                return qp[pi][:, ds(t * QCH - c0, QCH)]

            def emit_pv(job):
                # deferred PV + denominator matmuls for one group
                # (software pipelining: keeps the in-order PE queue from
                # head-of-line blocking on the exp/select chain of the group)
                (t, j0, gn, nj, e_sb, o_ps, db_ps, den_blk,
                 den_first, den_last) = job
                for d in range(gn):
                    j = j0 + d
                    dd = j - 4 * t
                    off = max(dd, 0) * P
                    nc.tensor.matmul(
                        o_ps[:, ds(off, QCH - off)],
                        v_tile(j),
                        e_sb[:, ds(d * QCH + off, QCH - off)],
                        start=(j == 0), stop=(j == nj - 1))
                if den_blk is not None:
                    nc.tensor.matmul(db_ps, ones_sq, den_blk,
                                     start=den_first, stop=den_last)
                if j0 + gn == nj:      # last group: flush chunk outputs
                    out_sb = spool.tile([P, QCH], F32, tag="osb",
                                        name=f"osb{t}")
                    den_sb = spool.tile([1, QCH], F32, tag="den",
                                        name=f"den{t}")
                    if t == CHUNK_ORDER[-1]:   # tail: split across engines
                        nc.scalar.copy(out_sb, o_ps)
                        nc.vector.tensor_copy(den_sb, db_ps[0:1, :])
                    else:
                        nc.vector.tensor_copy(out_sb, o_ps)
                        nc.vector.tensor_copy(den_sb, db_ps[0:1, :])
                    nc.sync.dma_start(outT[:, ts(t, QCH)], out_sb)
                    nc.sync.dma_start(den[:, ts(t, QCH)], den_sb)

            pv_pending = []        # deferred 2 groups: keeps the in-order
            for t in CHUNK_ORDER:  # PE queue's S matmuls ahead of PVs
                                   # that wait on the gpsimd select
                nj = 4 * (t + 1)          # causal: k-tiles 0..4t+3
                q_sl = q_chunk(t)
                o_ps = po_pool.tile([P, QCH], F32, tag="o")
                db_ps = pd_pool.tile([P, QCH], F32, tag="db")
                den_carry = None
                den_count = 0

                groups = []
                j0 = 0
                while j0 < nj:
                    gn = min(GROUP, nj - j0)
                    groups.append((j0, gn))
                    j0 += gn

                for (j0, gn) in groups:
                    s_ps = ps_pool.tile([P, gn * QCH], F32, tag="s",
                                        padded_shape=[P, GROUP * QCH])
                    for d in range(gn):
                        j = j0 + d
                        dd = j - 4 * t
                        off = max(dd, 0) * P   # fully-masked column prefix
                        nc.tensor.matmul(
                            s_ps[:, ds(d * QCH + off, QCH - off)],
                            k_tile(j), q_sl[:, ds(off, QCH - off)],
                            start=True, stop=True)
                    e_sb = epool.tile([P, gn * QCH], BF16, tag="e",
                                      padded_shape=[P, GROUP * QCH])
                    nc.scalar.activation(e_sb, s_ps, exp_fn, scale=SCALE)

                    # causal mask on diagonal tiles (j in [4t, 4t+4)):
                    # keep where qcol - k - 128*dd >= 0, else fill 0.
                    # Covers the skipped S prefix too (exp of stale PSUM).
                    for d in range(gn):
                        j = j0 + d
                        dd = j - 4 * t
                        if dd >= 0:
                            w = (dd + 1) * P
                            reg = e_sb[:, ds(d * QCH, w)]
                            nc.gpsimd.affine_select(
                                out=reg, in_=reg, compare_op=is_ge,
                                fill=0.0, base=-dd * P, pattern=[[1, w]],
                                channel_multiplier=-1)

                    # denominator partials: sum blocks on DVE (bf16 2x
                    # adds), chaining across pairs of groups; one all-ones
                    # matmul per pair reduces over partitions into db
                    gidx = j0 // GROUP
                    chain = den_carry if gidx % 2 == 1 else None
                    if gn == 1 and chain is None:
                        den_blk = e_sb[:, :QCH]
                    else:
                        qacc = qpool.tile([P, QCH], BF16, tag="qacc")
                        first2 = (chain if chain is not None
                                  else e_sb[:, ts(1, QCH)])
                        nc.vector.tensor_add(qacc, e_sb[:, ts(0, QCH)],
                                             first2)
                        for d in range(1 if chain is not None else 2, gn):
                            nc.vector.tensor_add(qacc, qacc,
                                                 e_sb[:, ts(d, QCH)])
                        den_blk = qacc
                    if gidx % 2 == 0 and j0 + gn < nj:
                        den_carry = den_blk      # defer to next group
                        den_blk = None
                    else:
                        den_carry = None

                    if len(pv_pending) >= 2:
                        emit_pv(pv_pending.pop(0))
                    den_first = den_blk is not None and den_count == 0
                    den_last = j0 + gn == nj
                    if den_blk is not None:
                        den_count += 1
                    pv_pending.append((t, j0, gn, nj, e_sb, o_ps, db_ps,
                                       den_blk, den_first, den_last))

            for job in pv_pending:
                emit_pv(job)

    nc.compile()
    return nc


def _get_nc():
    if not _nc_cache:
        _nc_cache.append(_build())
    return _nc_cache[0]


def _prep(query, key, value):
    B, H, W, C = query.shape
    CV = value.shape[-1]
    n = H * W
    q = (np.asarray(query, np.float32).reshape(B, n, C).transpose(0, 2, 1)
         .astype(ml_dtypes.bfloat16))
    q = np.ascontiguousarray(q)
    k = np.ascontiguousarray(
        np.asarray(key, np.float32).reshape(B, n, C).transpose(0, 2, 1)
        .astype(ml_dtypes.bfloat16))
    # vT[b, p, 128j+c] = v[b, 128j+p, c]: k-within-tile on partitions, so a
    # [128, 128] SBUF slice is directly the PV weight tile, and the HBM
    # lines are long and contiguous (8 KB per partition row)
    v = (np.asarray(value, np.float32).reshape(B, n // P, P, CV)
         .transpose(0, 2, 1, 3).reshape(B, P, n // P * CV)
         .astype(ml_dtypes.bfloat16))
    v = np.ascontiguousarray(v)
    return q, k, v


def kernel(query, key, value):
    B, H, W, C = query.shape
    CV = value.shape[-1]
    n = H * W
    q, k, v = _prep(query, key, value)

    nc = _get_nc()
    in_maps = [{"qT": q[b], "kT": k[b], "vT": v[b]} for b in range(B)]
    res = run_bass_kernel_spmd(nc, in_maps, core_ids=list(range(N_CORES)))

    out = np.empty((B, n, CV), np.float32)
    for b in range(B):
        oT = res.results[b]["outT"]          # [128, 4096] unnormalized O^T
        dn = res.results[b]["den"]           # [1, 4096]
        out[b] = (oT / dn).T
    return out.reshape(B, H, W, CV)
